# revision 4
# baseline (speedup 1.0000x reference)
"""Single-head causal attention (B=8, T=2048, C=1024, H=128) on 8 TRN2 cores.

Data-parallel over batch: core b computes attention for x[b].

Optimized pipeline (vs the f32r identity-transpose baseline):
  * While x streams in (~9 us, HBM-bound), the PE now runs FIVE
    concurrent projection accumulations — t-chunk 0's q/k/v (psA) plus
    t-chunk 1's q/k borrowed into the score banks (psS) — consuming each
    x chunk with 5 matmuls as it lands (~85% PE coverage of the stream
    window).  t-chunk 1's v runs right after from resident x.
  * PSUM->SBUF bias-add copybacks split into 256-wide halves so bank
    drains at phase boundaries stall the next accumulation ~350 ns
    instead of ~700.
  * x loads as 16 half-chunks, first t-half of every cb first: the
    stream-window projections only touch t < 1024, and a 256KB chunk's
    completion semaphore fires ~0.7us after gen instead of ~1.4, so the
    first matmul starts ~3us earlier.  y stores ride the sync queue
    (idle after the consts).
  * Otherwise v7: fp16 everywhere (FWL, 1 cyc/row), host-transposed x
    on gpsimd SWDGE, weights+consts on sync, schedule 0,1,3,2 with
    superblock 3's score/exp tiles spread across the kernel to keep the
    serial ACT exp drain off the tail.

Math identical to the baseline: Q^T/K^T/V^T = W^T x^T + b on the PE;
S^T = K^T.T @ Q^T puts softmax's k-reduction on the PSUM partition dim;
P = exp(S^T * scale) with triangular masking; a ones-column in V yields
softmax row sums inside the same accumulated PV matmul; normalization is
a reciprocal multiply on copyback.  No max-subtract (|scores| <= ~4).
"""

import numpy as np

import concourse.bass as bass
import concourse.mybir as mybir
import concourse.tile as tile
from concourse import bacc
from concourse.bass_utils import run_bass_kernel_spmd

B, T, C, H = 8, 2048, 1024, 128
P = 128
NCB = C // P  # 8 contraction chunks for the projections
NTB = T // P  # 16 token blocks
TCH = 512  # projection t-chunk width (one PSUM bank)
NTCH = T // TCH  # 4
QSB = 512  # query superblock width for attention
NQSB = T // QSB  # 4
VF = 132  # free width of the [V | 1 | 0-pad] tile
F32 = mybir.dt.float32
F16 = mybir.dt.float16
SCALE = float(C) ** -0.5

N_CORES = 8


def build_program():
    nc = bacc.Bacc(
        "TRN2",
        target_bir_lowering=False,
        debug=False,
        enable_asserts=False,
        num_devices=N_CORES,
    )

    xt_d = nc.dram_tensor("xt", (C, T), F16, kind="ExternalInput").ap()
    w_d = {
        nm: nc.dram_tensor(f"w{nm}", (P, NCB, H), F16, kind="ExternalInput").ap()
        for nm in ("q", "k", "v")
    }
    b_d = {
        nm: nc.dram_tensor(f"b{nm}", (P, 1), F32, kind="ExternalInput").ap()
        for nm in ("q", "k", "v")
    }
    ident_d = nc.dram_tensor("ident", (P, P), F16, kind="ExternalInput").ap()
    utri_d = nc.dram_tensor("utri", (P, P), F16, kind="ExternalInput").ap()
    y_d = nc.dram_tensor("y", (T, H), F32, kind="ExternalOutput").ap()

    with tile.TileContext(nc) as tc:
        with (
            tc.tile_pool(name="consts", bufs=1) as consts,
            tc.tile_pool(name="big", bufs=1) as big_pool,
            tc.tile_pool(name="ptile", bufs=30) as p_pool,
            tc.tile_pool(name="outs", bufs=4) as out_pool,
            tc.tile_pool(name="psA", bufs=3, space="PSUM") as psA,
            tc.tile_pool(name="psS", bufs=2, space="PSUM") as psS,
            tc.tile_pool(name="psB", bufs=2, space="PSUM") as psB,
            tc.tile_pool(name="psC", bufs=1, space="PSUM") as psC,
        ):
            # Weights ride sync ahead of everything (needed by the first
            # projection); gpsimd carries only the x stream + y stores.
            w_sb, b_sb = {}, {}
            for nm in ("q", "k", "v"):
                w_sb[nm] = consts.tile([P, NCB, H], F16, tag=f"w{nm}", name=f"w{nm}")
                nc.sync.dma_start(w_sb[nm], w_d[nm])
            ident = consts.tile([P, P], F16, tag="ident")
            nc.sync.dma_start(ident, ident_d)
            utri = consts.tile([P, P], F16, tag="utri")
            nc.sync.dma_start(utri, utri_d)
            for nm in ("q", "k", "v"):
                b_sb[nm] = consts.tile([P, 1], F32, tag=f"b{nm}", name=f"b{nm}")
                nc.sync.dma_start(b_sb[nm], b_d[nm])

            xT = big_pool.tile([P, NCB, T], F16, tag="xT")
            qT = big_pool.tile([P, T], F16, tag="qT")
            kT = big_pool.tile([P, T], F16, tag="kT")
            vT = big_pool.tile([P, T], F16, tag="vT")
            v2 = big_pool.tile([P, NTB, VF], F16, tag="v2")
            nc.vector.memset(v2[:, :, H + 1 :], 0.0)
            nc.vector.memset(v2[:, :, H : H + 1], 1.0)

            # PE warmup on a zeroed tile (ready ~6us, no DMA dep): a dozen
            # matmuls keep the HAM activity window busy through the DMA
            # head, so the stream-window projections run at 2.4 GHz.
            warmsrc = big_pool.tile([P, QSB], F16, tag="warmsrc")
            nc.vector.memset(warmsrc, 1.0)
            for i in range(12):
                wm = psB.tile([P, QSB], F32, tag="B", name=f"warm{i}")
                nc.tensor.matmul(
                    wm, warmsrc[:, :P], warmsrc, start=True, stop=True
                )

            # x (host-transposed) on gpsimd SWDGE: 16 half-chunks, first
            # t-half of every cb first — the stream window needs only
            # t < 1024, and smaller chunks land (and fire sems) sooner.
            XH = T // 2
            for th in range(2):
                hsl = slice(th * XH, (th + 1) * XH)
                for cb in range(NCB):
                    if th == 0 and cb == 0:
                        # cb0's first half in two quarters: the stream
                        # window's t-chunk-0 matmuls need only t < 512,
                        # so the first 128KB quarter unblocks the PE
                        # ~2us sooner.
                        nc.gpsimd.dma_start(
                            xT[:, 0, 0:TCH], xt_d[0:P, 0:TCH]
                        )
                        nc.gpsimd.dma_start(
                            xT[:, 0, TCH:XH], xt_d[0:P, TCH:XH]
                        )
                        continue
                    nc.gpsimd.dma_start(
                        xT[:, cb, hsl], xt_d[cb * P : (cb + 1) * P, hsl]
                    )

            proj = {"q": qT, "k": kT, "v": vT}

            def bias_add(dst, tsl, ps, nm):
                # halved so the PSUM bank frees in two ~350ns drains
                mid = (tsl.start + tsl.stop) // 2
                w = ps.shape[-1] // 2
                nc.vector.tensor_scalar_add(
                    dst[:, tsl.start : mid], ps[:, :w], b_sb[nm]
                )
                nc.vector.tensor_scalar_add(
                    dst[:, mid : tsl.stop], ps[:, w:], b_sb[nm]
                )

            def emit_vnat(tch):
                for tb in range(tch * (TCH // P), (tch + 1) * (TCH // P)):
                    ps = psC.tile([P, P], F16, tag="C")
                    nc.tensor.transpose(ps, vT[:, tb * P : (tb + 1) * P], ident)
                    nc.vector.tensor_copy(v2[:, tb, :P], ps)

            def emit_proj(tch, names=("q", "k", "v"), pool=None):
                pool = pool or psA
                tsl = slice(tch * TCH, (tch + 1) * TCH)
                ps3 = {
                    nm: pool.tile(
                        [P, TCH], F32, tag=pool.name[-1], name=f"ps_{nm}{tch}"
                    )
                    for nm in names
                }
                for cb in range(NCB):
                    for nm in names:
                        nc.tensor.matmul(
                            ps3[nm],
                            w_sb[nm][:, cb, :],
                            xT[:, cb, tsl],
                            start=(cb == 0),
                            stop=(cb == NCB - 1),
                        )
                for nm in names:
                    bias_add(proj[nm], tsl, ps3[nm], nm)

            def emit_scores(qs, kbs, p_tiles):
                for kb in kbs:
                    j0 = kb - qs * (QSB // P)  # first valid 128-col block
                    off = 0 if j0 <= 0 else j0 * P
                    ps = psS.tile([P, QSB], F32, tag="S")
                    nc.tensor.matmul(
                        ps[:, off:],
                        kT[:, kb * P : (kb + 1) * P],
                        qT[:, qs * QSB + off : (qs + 1) * QSB],
                        start=True,
                        stop=True,
                    )
                    pt = p_pool.tile([P, QSB], F16, tag="P")
                    e0 = max(j0, 0) * P
                    nc.scalar.activation(
                        pt[:, e0:],
                        ps[:, e0:],
                        mybir.ActivationFunctionType.Exp,
                        scale=SCALE,
                    )
                    if j0 >= 0:
                        nc.vector.tensor_tensor(
                            pt[:, j0 * P : (j0 + 1) * P],
                            pt[:, j0 * P : (j0 + 1) * P],
                            utri,
                            mybir.AluOpType.mult,
                        )
                    p_tiles[kb] = pt

            def emit_pv(qs, p_tiles):
                for j in range(QSB // P):
                    qb = qs * (QSB // P) + j
                    po = psB.tile([P, VF], F32, tag="B")
                    for kb in range(qb + 1):
                        nc.tensor.matmul(
                            po,
                            p_tiles[kb][:, j * P : (j + 1) * P],
                            v2[:, kb, :],
                            start=(kb == 0),
                            stop=(kb == qb),
                        )
                    rec = out_pool.tile([P, 1], F32, tag="rec")
                    nc.vector.reciprocal(rec, po[:, H : H + 1])
                    ot = out_pool.tile([P, H], F32, tag="out")
                    nc.vector.tensor_scalar_mul(ot, po[:, :H], rec)
                    nc.sync.dma_start(y_d[qb * P : (qb + 1) * P, :], ot)

            pt3 = {}  # superblock 3's P tiles, built incrementally
            pts = {}

            # Stream window: tch0 q/k/v (psA) + tch1 q/k (psS) — five
            # matmuls per arriving x chunk.
            tsl0 = slice(0, TCH)
            tsl1 = slice(TCH, 2 * TCH)
            psQKV = {
                nm: psA.tile([P, TCH], F32, tag="A", name=f"ps_{nm}0")
                for nm in ("q", "k", "v")
            }
            psQK1 = {
                nm: psS.tile([P, TCH], F32, tag="S", name=f"ps_{nm}1")
                for nm in ("q", "k")
            }
            for cb in range(NCB):
                for nm in ("q", "k", "v"):
                    nc.tensor.matmul(
                        psQKV[nm],
                        w_sb[nm][:, cb, :],
                        xT[:, cb, tsl0],
                        start=(cb == 0),
                        stop=(cb == NCB - 1),
                    )
                for nm in ("q", "k"):
                    nc.tensor.matmul(
                        psQK1[nm],
                        w_sb[nm][:, cb, :],
                        xT[:, cb, tsl1],
                        start=(cb == 0),
                        stop=(cb == NCB - 1),
                    )
            for nm in ("q", "k", "v"):
                bias_add(proj[nm], tsl0, psQKV[nm], nm)
            for nm in ("q", "k"):
                bias_add(proj[nm], tsl1, psQK1[nm], nm)

            emit_vnat(0)
            # attention superblock 0
            pts[0] = {}
            emit_scores(0, range(4), pts[0])
            emit_pv(0, pts[0])
            # finish t-chunk 1 (v only) then full superblock 1
            emit_proj(1, names=("v",))
            emit_vnat(1)
            pts[1] = {}
            emit_scores(1, range(8), pts[1])
            emit_pv(1, pts[1])
            # proj 3 -> superblock 3 scores for kT chunks 0,1,3
            emit_proj(3)
            emit_vnat(3)
            emit_scores(3, [0, 1, 2, 3, 4, 5, 6, 7, 12, 13, 14, 15], pt3)
            # proj 2 -> superblock 2 + the rest of superblock 3
            emit_proj(2)
            emit_vnat(2)
            pts[2] = {}
            emit_scores(2, range(12), pts[2])
            emit_scores(3, [8, 9, 10, 11], pt3)
            emit_pv(2, pts[2])
            emit_pv(3, pt3)

    nc.compile()
    return nc


_NC_CACHE = {}


def _get_program():
    if "nc" not in _NC_CACHE:
        _NC_CACHE["nc"] = build_program()
    return _NC_CACHE["nc"]


def make_in_maps(x, Wq, bq, Wk, bk, Wv, bv):
    f32 = lambda a: np.ascontiguousarray(np.asarray(a, dtype=np.float32))

    def warr(w):
        # [C, H] -> [P, NCB, H]: partition p holds chunks w[cb*128 + p, :]
        w = np.asarray(w, dtype=np.float16).reshape(NCB, P, H)
        return np.ascontiguousarray(w.transpose(1, 0, 2))

    ident = np.eye(P, dtype=np.float16)
    utri = np.triu(np.ones((P, P), dtype=np.float16))
    common = {
        "wq": warr(Wq),
        "wk": warr(Wk),
        "wv": warr(Wv),
        "bq": f32(bq).reshape(P, 1),
        "bk": f32(bk).reshape(P, 1),
        "bv": f32(bv).reshape(P, 1),
        "ident": ident,
        "utri": utri,
    }
    xt = np.asarray(x, dtype=np.float16).transpose(0, 2, 1)  # [B, C, T]
    xt = np.ascontiguousarray(xt)
    return [dict(common, xt=xt[b]) for b in range(N_CORES)]


def kernel(x, Wq, bq, Wk, bk, Wv, bv):
    nc = _get_program()
    in_maps = make_in_maps(x, Wq, bq, Wk, bk, Wv, bv)
    res = run_bass_kernel_spmd(nc, in_maps, core_ids=list(range(N_CORES)))
    return np.stack([res.results[b]["y"] for b in range(N_CORES)], axis=0)
